# revision 13
# baseline (speedup 1.0000x reference)
"""MobiuAttention Trainium2 kernel (8 NeuronCores, SPMD), v2 bf16.

Sharding: core i handles (batch b = i//2, head-group g = i%2) -> 8 local heads.
Per core: bf16 projections, hoisted complexity sensor (tanh/ln batched so the
activation table stays put), chunked linear-attention recurrence (chunk C=128,
log-space cumulative decay, head-PAIR packed on 128 partitions, bf16 matmuls),
o_proj partial with the local head-slice of o_w (bf16 out). Host sums the two
partial y's per batch in fp32.
"""
import sys
sys.path.insert(0, '/opt/trn_rl_repo')

import numpy as np
import ml_dtypes
import bass_rust
import concourse.bass as bass
import concourse.mybir as mybir
import concourse.tile as tile
from concourse.bass_utils import run_bass_kernel_spmd
from concourse.masks import make_identity, make_upper_triangular

F32 = mybir.dt.float32
BF16 = mybir.dt.bfloat16
AL = mybir.AluOpType
AF = mybir.ActivationFunctionType

B, T, D, H, E = 4, 2048, 1024, 16, 64
DH = D // 4          # 256 sensor hidden
HL = 8               # heads per core
NP = HL // 2         # 4 head pairs
DL = HL * E          # 512 local head dim
SC = 4               # superchunks
TC = T // SC         # 512 tokens per superchunk
C = 128              # recurrence chunk
NT = TC // C         # 4 chunks per superchunk
NCH = T // C         # 16 chunks total
NDT = D // 128       # 8 contraction tiles
LOGCLIP = float(np.log(0.9995))

SEQ_ENGINES = {mybir.EngineType.PE, mybir.EngineType.DVE, mybir.EngineType.Activation,
               mybir.EngineType.Pool, mybir.EngineType.SP}


def _split_multiwait(nc, max_waits=1):
    """Walrus encodes at most one sync-wait per instruction; hoist extra
    waits onto single-wait NOPs just before, on the same in-order sequencer."""
    for f in nc.m.functions:
        for bb in f.blocks:
            changed = False
            newlist = []
            for inst in bb.instructions:
                si = inst.sync_info
                if (si is not None and len(si.on_wait) > max_waits
                        and inst.engine in SEQ_ENGINES):
                    waits = list(si.on_wait)
                    for w in waits[:-1]:
                        nop = mybir.InstNoOp(name=nc.get_next_instruction_name(),
                                             ins=[], outs=[])
                        nop.engine = inst.engine
                        nop.sync_info = bass_rust.SyncInfo(on_wait=[w], on_update=[])
                        newlist.append(nop)
                        nc.register_instruction(nop)
                    inst.sync_info = bass_rust.SyncInfo(
                        on_wait=[waits[-1]], on_update=list(si.on_update))
                    changed = True
                newlist.append(inst)
            if changed:
                bb.instructions = newlist


def _build():
    nc = bass.Bass(trn_type="TRN2", num_devices=8)
    xT_d = nc.dram_tensor("xT", [128, NDT * T], BF16, kind="ExternalInput")
    wq_d = nc.dram_tensor("wq", [128, NDT * DL], BF16, kind="ExternalInput")
    wk_d = nc.dram_tensor("wk", [128, NDT * DL], BF16, kind="ExternalInput")
    wv_d = nc.dram_tensor("wv", [128, NDT * DL], BF16, kind="ExternalInput")
    wo_d = nc.dram_tensor("wo", [128, 4 * D], BF16, kind="ExternalInput")
    cs1_d = nc.dram_tensor("cs1", [128, NDT * DH], BF16, kind="ExternalInput")
    cs2_d = nc.dram_tensor("cs2", [128, 2 * HL], BF16, kind="ExternalInput")
    b1_d = nc.dram_tensor("b1", [128, 2], F32, kind="ExternalInput")
    b2_d = nc.dram_tensor("b2", [128, HL], F32, kind="ExternalInput")
    lb_d = nc.dram_tensor("lb", [128, DL], F32, kind="ExternalInput")
    y_d = nc.dram_tensor("y", [T, D], BF16, kind="ExternalOutput")

    with tile.TileContext(nc) as tc:
        with tc.tile_pool(name="wpool", bufs=1) as wpool, \
             tc.tile_pool(name="cpool", bufs=1) as cpool, \
             tc.tile_pool(name="state", bufs=1) as state, \
             tc.tile_pool(name="xpool", bufs=1) as xpool, \
             tc.tile_pool(name="hpool", bufs=1) as hpool, \
             tc.tile_pool(name="qkpool", bufs=1) as qkpool, \
             tc.tile_pool(name="vpool", bufs=1) as vpool, \
             tc.tile_pool(name="otpool", bufs=2) as otpool, \
             tc.tile_pool(name="ypool", bufs=3) as ypool, \
             tc.tile_pool(name="rec", bufs=3) as rec, \
             tc.tile_pool(name="small", bufs=4) as small, \
             tc.tile_pool(name="psA", bufs=2, space="PSUM") as psA, \
             tc.tile_pool(name="psR", bufs=1, space="PSUM") as psR, \
             tc.tile_pool(name="psP", bufs=2, space="PSUM") as psP:

            # ---- constants ----
            identf = cpool.tile([128, 128], F32)
            make_identity(nc, identf[:])
            identb = cpool.tile([128, 128], BF16)
            nc.vector.tensor_copy(identb[:], identf[:])
            tri = cpool.tile([128, 128], F32)
            make_upper_triangular(nc, tri[:], val=1.0, diag=True)
            tri_u8 = cpool.tile([128, 128], mybir.dt.uint8)
            nc.vector.tensor_copy(tri_u8[:], tri[:])
            # persistent atm tiles: masked (strictly-lower) region stays 0 from
            # this one-time init; copy_predicated only ever writes upper lanes
            atm_t = []
            for ji in range(4):
                a = cpool.tile([128, 128], BF16, tag=f"atm{ji}", name=f"atm{ji}")
                nc.vector.memset(a[:], 0.0)
                atm_t.append(a)
            z128 = cpool.tile([128, 128], F32)
            nc.vector.memset(z128[:], 0.0)
            c11 = cpool.tile([128, 1], F32)
            nc.vector.memset(c11[:], 1.1)

            # ---- weights ----
            wq = wpool.tile([128, NDT * DL], BF16)
            nc.sync.dma_start(wq[:], wq_d[:])
            wk = wpool.tile([128, NDT * DL], BF16)
            nc.sync.dma_start(wk[:], wk_d[:])
            wv = wpool.tile([128, NDT * DL], BF16)
            nc.sync.dma_start(wv[:], wv_d[:])
            wo = wpool.tile([128, 4 * D], BF16)
            nc.sync.dma_start(wo[:], wo_d[:])
            cs1 = wpool.tile([128, NDT * DH], BF16)
            nc.sync.dma_start(cs1[:], cs1_d[:])
            cs2 = wpool.tile([128, 2 * HL], BF16)
            nc.sync.dma_start(cs2[:], cs2_d[:])
            b1 = wpool.tile([128, 2], F32)
            nc.sync.dma_start(b1[:], b1_d[:])
            b2 = wpool.tile([128, HL], F32)
            nc.sync.dma_start(b2[:], b2_d[:])
            lb = wpool.tile([128, DL], F32)
            nc.sync.dma_start(lb[:], lb_d[:])

            # ---- x resident (per dt block so consumers gate per-block) ----
            xb = []
            for dt in range(NDT):
                xt = xpool.tile([128, T], BF16, tag=f"x{dt}", name=f"x{dt}")
                nc.sync.dma_start(xt[:], xT_d[:, dt * T:(dt + 1) * T])
                xb.append(xt)

            # ---- per-pair recurrent state [ (h0 e | h1 e), f ] ----
            S = []
            for mo in range(NP):
                sh = state.tile([128, 64], F32, tag=f"S{mo}", name=f"S{mo}")
                nc.vector.memset(sh[:], 0.0)
                S.append(sh)

            # ================= Phase A: sensor (all T) =================
            # hid tiles: [128 hid-dim, T] bf16, tanh applied (Tanh table)
            hid = [hpool.tile([128, T], BF16, tag=f"hid{hb}", name=f"hid{hb}")
                   for hb in range(2)]
            for hb in range(2):
                for scb in range(SC):
                    pp = psP.tile([128, TC], F32, tag="proj")
                    for dt in range(NDT):
                        nc.tensor.matmul(
                            pp[:],
                            cs1[:, dt * DH + hb * 128: dt * DH + (hb + 1) * 128],
                            xb[dt][:, scb * TC:(scb + 1) * TC],
                            start=(dt == 0), stop=(dt == NDT - 1))
                    nc.scalar.activation(hid[hb][:, scb * TC:(scb + 1) * TC],
                                         pp[:], AF.Tanh, bias=b1[:, hb:hb + 1])

            # z = hid @ cs2 per chunk -> [128 t, HL]; th_all collects tanh(z/2)
            th_all = hpool.tile([128, NCH * HL], F32, tag="th", name="th_all")
            for ch in range(NCH):
                zp = psA.tile([128, 128], F32, tag="lamT")
                for hb in range(2):
                    nc.tensor.matmul(
                        zp[:, 0:HL],
                        hid[hb][:, ch * C:(ch + 1) * C],
                        cs2[:, hb * HL:(hb + 1) * HL],
                        start=(hb == 0), stop=(hb == 1))
                zs = small.tile([128, HL], F32, tag="zs")
                nc.vector.tensor_add(zs[:], zp[:, 0:HL], b2[:])
                nc.scalar.activation(th_all[:, ch * HL:(ch + 1) * HL],
                                     zs[:], AF.Tanh, scale=0.5)
            # u = ln(1.1 + 0.1*th)  (exact sigmoid fold); one Ln op
            u_all = hpool.tile([128, NCH * HL], F32, tag="u", name="u_all")
            nc.scalar.activation(u_all[:], th_all[:], AF.Ln, scale=0.1,
                                 bias=c11[:])

            # ============ Phase B1: all q/k/v projections (PE-dense) ============
            q_et = [[None] * NP for _ in range(SC)]
            k_et = [[None] * NP for _ in range(SC)]
            v_bf = [[None] * NT for _ in range(SC)]
            for sc in range(SC):
                for name, w, dst in (("q", wq, q_et), ("k", wk, k_et)):
                    for mo in range(NP):
                        pp = psP.tile([128, TC], F32, tag="proj")
                        for dt in range(NDT):
                            nc.tensor.matmul(
                                pp[:],
                                w[:, dt * DL + mo * 128: dt * DL + (mo + 1) * 128],
                                xb[dt][:, sc * TC:(sc + 1) * TC],
                                start=(dt == 0), stop=(dt == NDT - 1))
                        sb = qkpool.tile([128, TC], BF16, tag=f"{name}{sc}{mo}",
                                         name=f"{name}_{sc}_{mo}")
                        if name == "q":
                            nc.vector.tensor_copy(sb[:], pp[:])
                        else:
                            nc.scalar.copy(sb[:], pp[:])
                        dst[sc][mo] = sb
                for tt in range(NT):
                    pp = psP.tile([128, DL], F32, tag="proj")
                    for dt in range(NDT):
                        nc.tensor.matmul(
                            pp[:],
                            xb[dt][:, sc * TC + tt * C: sc * TC + (tt + 1) * C],
                            wv[:, dt * DL:(dt + 1) * DL],
                            start=(dt == 0), stop=(dt == NDT - 1))
                    vb = vpool.tile([128, DL], BF16, tag=f"v{sc}{tt}",
                                    name=f"v_{sc}_{tt}")
                    nc.scalar.copy(vb[:], pp[:])
                    v_bf[sc][tt] = vb

            # ============ Phase B2: recurrence + o_proj ============
            for sc in range(SC):
                OT = [otpool.tile([128, TC], BF16, tag=f"ot{mo}",
                                  name=f"OT{mo}_{sc}") for mo in range(NP)]
                for tt in range(NT):
                    ch = sc * NT + tt
                    for mo in range(NP):
                        q_p = q_et[sc][mo][:, tt * C:(tt + 1) * C]
                        k_p = k_et[sc][mo][:, tt * C:(tt + 1) * C]
                        vbf_p = v_bf[sc][tt][:, mo * 128:(mo + 1) * 128]

                        # log-lambda [t, (2x64 e)] (Pool), transpose, scan
                        lam = rec.tile([128, 128], F32, tag="lam")
                        for j in range(2):
                            h = 2 * mo + j
                            nc.gpsimd.tensor_scalar(
                                lam[:, j * 64:(j + 1) * 64],
                                lb[:, h * 64:(h + 1) * 64],
                                u_all[:, ch * HL + h: ch * HL + h + 1],
                                LOGCLIP, AL.add, AL.min)
                        lamT = psA.tile([128, 128], F32, tag="lamT")
                        nc.tensor.transpose(lamT[:], lam[:], identf[:])
                        L = rec.tile([128, 128], F32, tag="L")
                        nc.vector.tensor_tensor_scan(
                            L[:], lamT[:], z128[:], 0.0, AL.add, AL.add)

                        L127 = L[:, 127:128]
                        ccol = small.tile([128, 1], F32, tag="ccol")
                        nc.gpsimd.tensor_scalar_mul(ccol[:], L127, 0.5)
                        cneg = small.tile([128, 1], F32, tag="cneg")
                        nc.gpsimd.tensor_scalar_mul(cneg[:], L127, -0.5)
                        ec = small.tile([128, 1], F32, tag="ec")
                        nc.scalar.activation(ec[:], L127, AF.Exp, scale=0.5)
                        aend = small.tile([128, 1], F32, tag="aend")
                        nc.gpsimd.tensor_mul(aend[:], ec[:], ec[:])

                        eq = rec.tile([128, 128], F32, tag="eq")
                        nc.scalar.activation(eq[:], L[:], AF.Exp, bias=cneg[:])
                        ekc = rec.tile([128, 128], F32, tag="ekc")
                        nc.scalar.activation(ekc[:], L[:], AF.Exp, bias=ccol[:],
                                             scale=-1.0)

                        qt = rec.tile([128, 128], BF16, tag="qt")
                        nc.vector.tensor_mul(qt[:], q_p, eq[:])
                        kt = rec.tile([128, 128], BF16, tag="kt")
                        nc.vector.tensor_mul(kt[:], k_p, ekc[:])
                        kh = rec.tile([128, 128], BF16, tag="kh")
                        nc.gpsimd.tensor_scalar_mul(kh[:], kt[:], ec[:])

                        # K-hat pair transpose -> [t, (2x64 e)]
                        khT = psA.tile([128, 128], BF16, tag="at")
                        nc.tensor.transpose(khT[:], kh[:], identb[:])
                        khTs = rec.tile([128, 128], BF16, tag="khTs")
                        nc.vector.tensor_copy(khTs[:], khT[:])

                        # S_scaled (both heads)
                        ssc = rec.tile([128, 64], BF16, tag="ssc")
                        nc.gpsimd.tensor_scalar_mul(ssc[:], S[mo][:], ec[:])

                        # state delta for the pair (block-diagonal valid)
                        sd = psR.tile([128, 128], F32, tag="sd")
                        nc.tensor.matmul(sd[:], khTs[:], vbf_p,
                                         start=True, stop=True)

                        op = psR.tile([128, 128], F32, tag="outT")
                        for j in range(2):
                            sl = slice(j * 64, (j + 1) * 64)
                            at = psA.tile([128, 128], F32, tag="at")
                            nc.tensor.matmul(at[:], kt[sl, :], qt[sl, :],
                                             start=True, stop=True)
                            atm = atm_t[2 * j + ((sc * NT + tt) * NP + mo) % 2]
                            nc.vector.copy_predicated(atm[:], tri_u8[:], at[:])

                            nc.tensor.matmul(op[sl, :],
                                             vbf_p[:, j * 64:(j + 1) * 64],
                                             atm[:], start=True, stop=False)
                            nc.tensor.matmul(op[sl, :], ssc[sl, :], qt[sl, :],
                                             start=False, stop=True)
                            nc.vector.scalar_tensor_tensor(
                                S[mo][sl, :], S[mo][sl, :], aend[sl, :],
                                sd[sl, j * 64:(j + 1) * 64], AL.mult, AL.add)
                        nc.vector.tensor_copy(
                            OT[mo][:, tt * C:(tt + 1) * C], op[:])

                    # o_proj for this chunk
                    for no in range(2):
                        pp = psP.tile([128, 512], F32, tag="proj")
                        for mo in range(NP):
                            nc.tensor.matmul(
                                pp[:],
                                OT[mo][:, tt * C:(tt + 1) * C],
                                wo[:, mo * D + no * 512: mo * D + no * 512 + 512],
                                start=(mo == 0), stop=(mo == NP - 1))
                        ysb = ypool.tile([128, 512], BF16, tag="y")
                        nc.scalar.copy(ysb[:], pp[:])
                        nc.sync.dma_start(
                            y_d[sc * TC + tt * C: sc * TC + (tt + 1) * C,
                                no * 512:(no + 1) * 512],
                            ysb[:])
    _split_multiwait(nc)
    return nc


_NC = None

def _get_nc():
    global _NC
    if _NC is None:
        _NC = _build()
    return _NC


def _sigmoid(x):
    return 1.0 / (1.0 + np.exp(-x))


def kernel(x, q_w, k_w, v_w, o_w, cs_w1, cs_b1, cs_w2, cs_b2, decay_params):
    x = np.asarray(x, np.float32)
    nc = _get_nc()
    bf = ml_dtypes.bfloat16

    def wlay(wT_cols):  # [1024, M] -> [128, 8*M] (dt-major along free), bf16
        return np.ascontiguousarray(
            wT_cols.reshape(NDT, 128, wT_cols.shape[1]).transpose(1, 0, 2)
            .reshape(128, -1).astype(bf))

    qwT = np.asarray(q_w, np.float32).T
    kwT = np.asarray(k_w, np.float32).T
    vwT = np.asarray(v_w, np.float32).T
    owT = np.asarray(o_w, np.float32).T
    cs1T = np.asarray(cs_w1, np.float32).T      # [1024, 256]
    cs2T = np.asarray(cs_w2, np.float32).T      # [256, 16]
    lbase = np.log(_sigmoid(np.asarray(decay_params, np.float32)))  # [H, E]
    b1c = np.ascontiguousarray(np.asarray(cs_b1, np.float32).reshape(2, 128).T)

    in_maps = []
    for i in range(8):
        b, g = i // 2, i % 2
        hs = g * HL
        xT = x[b].T                                            # [1024, 2048]
        xTl = np.ascontiguousarray(
            xT.reshape(NDT, 128, T).transpose(1, 0, 2).reshape(128, NDT * T)
            .astype(bf))
        wo_loc = owT[hs * E:(hs + HL) * E, :]                  # [512, 1024]
        wol = np.ascontiguousarray(                            # [128, 4*1024]
            wo_loc.reshape(4, 128, D).transpose(1, 0, 2).reshape(128, 4 * D)
            .astype(bf))
        cs2l = np.ascontiguousarray(
            cs2T[:, hs:hs + HL].reshape(2, 128, HL).transpose(1, 0, 2)
            .reshape(128, 2 * HL).astype(bf))
        in_maps.append({
            "xT": xTl,
            "wq": wlay(qwT[:, hs * E:(hs + HL) * E]),
            "wk": wlay(kwT[:, hs * E:(hs + HL) * E]),
            "wv": wlay(vwT[:, hs * E:(hs + HL) * E]),
            "wo": wol,
            "cs1": wlay(cs1T),
            "cs2": cs2l,
            "b1": b1c,
            "b2": np.ascontiguousarray(
                np.broadcast_to(np.asarray(cs_b2, np.float32)[hs:hs + HL],
                                (128, HL))),
            "lb": np.ascontiguousarray(
                np.broadcast_to(lbase[hs:hs + HL].reshape(1, DL), (128, DL))),
        })

    res = run_bass_kernel_spmd(nc, in_maps, core_ids=list(range(8)))
    global LAST_RES
    LAST_RES = res
    y = np.empty((B, T, D), np.float32)
    for b in range(B):
        y[b] = (res.results[2 * b]["y"].astype(np.float32)
                + res.results[2 * b + 1]["y"].astype(np.float32))
    return y
